# revision 11
# baseline (speedup 1.0000x reference)
"""Trainium2 Bass kernel for nn_CrossAttention (B=8, N=4096, C=768, NH=8, 2 views).

Strategy: pure data-parallel over batch B (one batch element per NeuronCore).
Everything runs in "transposed space" (channels on SBUF partitions, tokens on
the free axis). Host stages activations pre-transposed as bf16 (halves HBM
traffic vs f32 — a pure layout/dtype staging choice).

Algebra (5 projections instead of 6):
  kd  = (k0 - k1) @ Wk.T          (subtract raw views first; projection is linear)
  vd  = (v0 - v1) @ Wk.T
  a0  = sigmoid(scale * sum_head(qhat * kd))        (2-view softmax == sigmoid)
  out = v1 @ W2.T + (a0 (.)head vd) @ Wp.T + bp,    W2 = Wp @ Wk  (host-folded)

The q/kd projections only feed the logits (errors are sigmoid-damped), so they
run in fp8e4 with DoubleRow perf mode (2 contraction rows per PE cell = 2x
throughput). Weights for those two projections are host-scaled by 256 to clear
the fp8e4 subnormal range; the 1/65536 compensation folds into the sigmoid
scale. The value path (vd / v1 / output proj) stays bf16.

Per-token attention over the 2 views needs only per-head segmented reductions
(hm mask matmul) and a per-head broadcast of a0 (sel matmul), both tiny on PE.
"""

from contextlib import ExitStack

import numpy as np
import ml_dtypes

import concourse.bass as bass
import concourse.mybir as mybir
import concourse.tile as tile
from concourse import bacc
from concourse.bass_utils import run_bass_kernel_spmd

B, N, C, NH, HD = 8, 4096, 768, 8, 96
P = 128
KO = C // P            # 6 channel chunks of 128
KO2 = KO // 2          # 3 double-row chunks of 256
BLK = 512              # tokens per block
NBLK = N // BLK        # 8 blocks per core
NCORES = 8
SCALE = float(HD) ** -0.5
W8SCALE = 256.0        # fp8 weight prescale (host); folded out of the sigmoid
SCALE_SIG = SCALE / (W8SCALE * W8SCALE)
F32 = mybir.dt.float32
BF16 = mybir.dt.bfloat16
FP8 = mybir.dt.float8e4
DR = mybir.MatmulPerfMode.DoubleRow

_STATE = {}


def _build_core_kernel(ctx, tc, aps, reps=1):
    nc = tc.nc

    consts = ctx.enter_context(tc.tile_pool(name="consts", bufs=1))
    inp = ctx.enter_context(tc.tile_pool(name="inp", bufs=2))
    att = ctx.enter_context(tc.tile_pool(name="att", bufs=2))
    # PSUM: 8 banks total. qk: rotating q/kd projection pairs + a0 broadcast
    # (4) | big: vd projection + output projection (3) | psl: logits (1).
    ps_qk = ctx.enter_context(tc.tile_pool(name="ps_qk", bufs=4, space="PSUM"))
    ps_big = ctx.enter_context(tc.tile_pool(name="ps_big", bufs=3, space="PSUM"))
    ps_l = ctx.enter_context(tc.tile_pool(name="ps_l", bufs=1, space="PSUM"))

    # Weights land as [P, KO(c_in), C(c_out)]; fp8 ones are host-prescaled
    # and split hi/lo (w ~= hi + lo, both fp8) so only the fp8 *activation*
    # quantization error survives — halves the logits noise for 2 extra
    # DoubleRow matmul groups per projection.
    w_sb = {}
    for wname, dt in (("wq8h", FP8), ("wq8l", FP8), ("wk8h", FP8),
                      ("wk8l", FP8), ("wkT", BF16), ("w2T", BF16),
                      ("wpT", BF16)):
        w_sb[wname] = consts.tile([P, KO, C], dt, tag=wname, name=wname)

    def _load_w(wname):
        nc.sync.dma_start(
            out=w_sb[wname][:],
            in_=aps[wname].rearrange("(ko p) o -> p ko o", p=P),
        )

    # Early consts: everything phase_a of block 0 touches.
    _load_w("wq8h")
    _load_w("wq8l")
    _load_w("wk8h")
    _load_w("wk8l")
    _load_w("wkT")
    bias_sb = consts.tile([P, KO], F32, tag="bias")
    hm_sb = consts.tile([P, KO, NH], BF16, tag="hm")
    sel_sb = consts.tile([NH, KO, P], BF16, tag="sel")
    nc.sync.dma_start(hm_sb[:], aps["hm"])

    def _load_late_consts():
        # consumed only by phase_b1/b2, which are emitted after this point
        _load_w("w2T")
        _load_w("wpT")
        nc.sync.dma_start(bias_sb[:], aps["bias"])
        nc.sync.dma_start(sel_sb[:], aps["sel"])

    r_in = {name: aps[name].rearrange("(ko p) n -> p ko n", p=P)
            for name in ("qT", "k0T", "k1T", "v0T", "v1T")}
    outT_r = aps["outT"].rearrange("(ko p) n -> p ko n", p=P)

    # Logits accumulator: single bank reused by every block; the tile
    # framework's writer-after-reader dep on the sigmoid keeps it correct
    # (the sigmoid of block b is emitted before block b+1's hm matmuls).
    psl = ps_l.tile([NH, BLK], F32, tag="psl", name="psl")

    def phase_a(blk):
        """Loads, fp8 q/kd projections + logits, bf16 vd projection, sigmoid."""
        ts = bass.ts(blk, BLK)
        q_in = inp.tile([P, KO, BLK], BF16, tag="q", name="q")
        nc.sync.dma_start(out=q_in[:], in_=r_in["qT"][:, :, ts])
        k0 = inp.tile([P, KO, BLK], BF16, tag="k0", name="k0")
        nc.sync.dma_start(out=k0[:], in_=r_in["k0T"][:, :, ts])
        k1 = inp.tile([P, KO, BLK], BF16, tag="k1", name="k1")
        nc.sync.dma_start(out=k1[:], in_=r_in["k1T"][:, :, ts])
        v0 = inp.tile([P, KO, BLK], BF16, tag="v0", name="v0")
        nc.sync.dma_start(out=v0[:], in_=r_in["v0T"][:, :, ts])
        # v1 is read by phase_b2(b), which is emitted after phase_a(b+2):
        # bufs=3 keeps the WAR dep acyclic (bufs=2 would deadlock PE).
        v1 = inp.tile([P, KO, BLK], BF16, tag="v1", name="v1", bufs=3)
        nc.sync.dma_start(out=v1[:], in_=r_in["v1T"][:, :, ts])

        q8 = att.tile([P, KO, BLK], FP8, tag="q8", name="q8")
        nc.scalar.copy(q8[:], q_in[:])
        kd8 = att.tile([P, KO, BLK], FP8, tag="kd8", name="kd8")
        nc.vector.tensor_sub(kd8[:], k0[:], k1[:])
        vd = att.tile([P, KO, BLK], BF16, tag="vd", name="vd")
        nc.vector.tensor_sub(vd[:], v0[:], v1[:])

        # fp8 DoubleRow q/kd projections; qkd = qhat .* kdhat (scaled 65536x).
        # kdhat bounces through SBUF (ACT copy): an engine op may read at
        # most one PSUM operand (NCC_IBVF027).
        qkd = att.tile([P, KO, BLK], BF16, tag="qkd", name="qkd")
        kdh = att.tile([P, KO, BLK], BF16, tag="kdh", name="kdh")
        for oc in range(KO):
            pq = ps_qk.tile([P, BLK], F32, tag="qk", name="pq")
            for i, wn in enumerate(("wq8h", "wq8l")):
                for k2 in range(KO2):
                    nc.tensor.matmul(
                        pq[:], w_sb[wn][:, bass.ts(k2, 2), bass.ts(oc, P)],
                        q8[:, bass.ts(k2, 2), :],
                        start=(i == 0 and k2 == 0),
                        stop=(i == 1 and k2 == KO2 - 1), perf_mode=DR,
                    )
            pk = ps_qk.tile([P, BLK], F32, tag="qk", name="pk")
            for i, wn in enumerate(("wk8h", "wk8l")):
                for k2 in range(KO2):
                    nc.tensor.matmul(
                        pk[:], w_sb[wn][:, bass.ts(k2, 2), bass.ts(oc, P)],
                        kd8[:, bass.ts(k2, 2), :],
                        start=(i == 0 and k2 == 0),
                        stop=(i == 1 and k2 == KO2 - 1), perf_mode=DR,
                    )
            nc.scalar.copy(kdh[:, oc, :], pk[:])
            nc.vector.tensor_mul(qkd[:, oc, :], pq[:], kdh[:, oc, :])

        # logits diff: psl[h, n] = sum_c qkd[c, n] over head h (= 65536*(l0-l1))
        for oc in range(KO):
            nc.tensor.matmul(
                psl[:], hm_sb[:, oc, :], qkd[:, oc, :],
                start=(oc == 0), stop=(oc == KO - 1),
            )

        # vd projection (bf16)
        vdh = att.tile([P, KO, BLK], BF16, tag="vdh", name="vdh")
        for oc in range(KO):
            pv = ps_big.tile([P, BLK], F32, tag="big", name="pv")
            for ko in range(KO):
                nc.tensor.matmul(
                    pv[:], w_sb["wkT"][:, ko, bass.ts(oc, P)], vd[:, ko, :],
                    start=(ko == 0), stop=(ko == KO - 1),
                )
            nc.scalar.copy(vdh[:, oc, :], pv[:])

        # a0 = sigmoid(scale * (l0 - l1)); emitted here (not b1) so the next
        # block's hm matmuls only wait on completed ACT work.
        a = att.tile([NH, BLK], BF16, tag="a", name="a")
        nc.scalar.activation(a[:], psl[:],
                             mybir.ActivationFunctionType.Sigmoid,
                             scale=SCALE_SIG)
        return blk, a, vdh, v1

    def phase_b1(state):
        """Per-head broadcast of a0 (PE) and weighted combine (DVE)."""
        blk, a, vdh, v1 = state
        y_in = att.tile([P, KO, BLK], BF16, tag="y", name="y")
        for oc in range(KO):
            bc = ps_qk.tile([P, BLK], F32, tag="qk", name="bc")
            nc.tensor.matmul(bc[:], sel_sb[:, oc, :], a[:],
                             start=True, stop=True)
            nc.vector.tensor_mul(y_in[:, oc, :], bc[:], vdh[:, oc, :])
        return blk, y_in, v1

    def phase_b2(state):
        """Output projection: v1 @ W2.T + y_in @ Wp.T + bias, store."""
        blk, y_in, v1 = state
        out_sb = att.tile([P, KO, BLK], BF16, tag="out", name="out_sb")
        for oc in range(KO):
            po = ps_big.tile([P, BLK], F32, tag="big", name="po")
            for ko in range(KO):
                nc.tensor.matmul(
                    po[:], w_sb["w2T"][:, ko, bass.ts(oc, P)], v1[:, ko, :],
                    start=(ko == 0), stop=False,
                )
            for ko in range(KO):
                nc.tensor.matmul(
                    po[:], w_sb["wpT"][:, ko, bass.ts(oc, P)], y_in[:, ko, :],
                    start=False, stop=(ko == KO - 1),
                )
            nc.vector.tensor_scalar_add(out_sb[:, oc, :], po[:],
                                        bias_sb[:, bass.ts(oc, 1)])
        nc.sync.dma_start(out=outT_r[:, :, bass.ts(blk, BLK)], in_=out_sb[:])

    # 3-stage software pipeline (same skew as the v1 kernel): per-block PE
    # order is ... A(b+2) | P-proj(b) | bc(b+1) ... so DVE/ACT latencies hide
    # under other blocks' matmuls.
    st_a = [phase_a(0)]
    _load_late_consts()
    st_a.append(phase_a(1))
    st_b = [phase_b1(st_a[0])]
    blocks = [(rep, blk) for rep in range(reps) for blk in range(NBLK)]
    for _, blk in blocks[2:]:
        st_a.append(phase_a(blk))
        phase_b2(st_b[-1])
        st_b.append(phase_b1(st_a[-2]))
    phase_b2(st_b[-1])
    st_b.append(phase_b1(st_a[-1]))
    phase_b2(st_b[-1])


def build_program(reps=1):
    nc = bacc.Bacc("TRN2", debug=False, target_bir_lowering=False)
    aps = {}
    for name in ("qT", "k0T", "k1T", "v0T", "v1T"):
        aps[name] = nc.dram_tensor(name, [C, N], BF16, kind="ExternalInput").ap()
    for name in ("wq8h", "wq8l", "wk8h", "wk8l"):
        aps[name] = nc.dram_tensor(name, [C, C], FP8, kind="ExternalInput").ap()
    for name in ("wkT", "w2T", "wpT"):
        aps[name] = nc.dram_tensor(name, [C, C], BF16, kind="ExternalInput").ap()
    aps["bias"] = nc.dram_tensor("bias", [P, KO], F32, kind="ExternalInput").ap()
    aps["hm"] = nc.dram_tensor("hm", [P, KO, NH], BF16, kind="ExternalInput").ap()
    aps["sel"] = nc.dram_tensor("sel", [NH, KO, P], BF16, kind="ExternalInput").ap()
    aps["outT"] = nc.dram_tensor("outT", [C, N], BF16, kind="ExternalOutput").ap()

    with tile.TileContext(nc) as tc, ExitStack() as ctx:
        _build_core_kernel(ctx, tc, aps, reps=reps)
    nc.compile()
    return nc


def _get_program():
    if "nc" not in _STATE:
        _STATE["nc"] = build_program()
    return _STATE["nc"]


def make_host_constants(bp):
    bf = ml_dtypes.bfloat16
    heads = np.arange(C) // HD                      # [C]
    bias = np.ascontiguousarray(
        np.asarray(bp, np.float32).reshape(KO, P).T)  # [P, KO]
    hm = np.zeros((C, NH), np.float32)
    for h in range(NH):
        hm[heads == h, h] = 1.0
    hm = np.ascontiguousarray(
        hm.reshape(KO, P, NH).transpose(1, 0, 2)).astype(bf)  # [P, KO, NH]
    sel = np.zeros((NH, C), np.float32)
    for h in range(NH):
        sel[h, heads == h] = 1.0
    sel = np.ascontiguousarray(
        sel.reshape(NH, KO, P)).astype(bf)           # [NH, KO, P]
    return bias, hm, sel


def make_in_maps(query, key, value, Wq, Wk, Wp, bp):
    bf = ml_dtypes.bfloat16
    f8 = ml_dtypes.float8_e4m3
    query = np.asarray(query, np.float32)
    key = np.asarray(key, np.float32)
    value = np.asarray(value, np.float32)
    Wq = np.asarray(Wq, np.float32)
    Wk = np.asarray(Wk, np.float32)
    Wp = np.asarray(Wp, np.float32)
    def split8(w):
        ws = np.ascontiguousarray(np.clip(w.T * W8SCALE, -240.0, 240.0))
        hi = ws.astype(f8)
        lo = (ws - hi.astype(np.float32)).astype(f8)
        return hi, lo

    wq8h, wq8l = split8(Wq)
    wk8h, wk8l = split8(Wk)
    wkT = np.ascontiguousarray(Wk.T).astype(bf)
    w2T = np.ascontiguousarray((Wp @ Wk).T).astype(bf)
    wpT = np.ascontiguousarray(Wp.T).astype(bf)
    bias, hm, sel = make_host_constants(bp)
    in_maps = []
    for b in range(NCORES):
        in_maps.append({
            "qT": np.ascontiguousarray(query[b].T).astype(bf),
            "k0T": np.ascontiguousarray(key[b, :, 0, :].T).astype(bf),
            "k1T": np.ascontiguousarray(key[b, :, 1, :].T).astype(bf),
            "v0T": np.ascontiguousarray(value[b, :, 0, :].T).astype(bf),
            "v1T": np.ascontiguousarray(value[b, :, 1, :].T).astype(bf),
            "wq8h": wq8h, "wq8l": wq8l, "wk8h": wk8h, "wk8l": wk8l,
            "wkT": wkT, "w2T": w2T, "wpT": wpT,
            "bias": bias, "hm": hm, "sel": sel,
        })
    return in_maps


def run(query, key, value, Wq, Wk, Wp, bp, trace=False, **trace_kwargs):
    nc = _get_program()
    in_maps = make_in_maps(query, key, value, Wq, Wk, Wp, bp)
    res = run_bass_kernel_spmd(nc, in_maps, list(range(NCORES)),
                               trace=trace, **trace_kwargs)
    out = np.stack([np.ascontiguousarray(r["outT"]).astype(np.float32).T
                    for r in res.results])
    return out, res


def kernel(query, key, value, Wq, Wk, Wp, bp):
    out, _ = run(query, key, value, Wq, Wk, Wp, bp)
    return out


# revision 15
# speedup vs baseline: 1.8581x; 1.8581x over previous
"""Trainium2 Bass kernel for nn_CrossAttention (B=8, N=4096, C=768, NH=8, 2 views).

Strategy: pure data-parallel over batch B (one batch element per NeuronCore).
Everything runs in "transposed space" (channels on SBUF partitions, tokens on
the free axis). Host stages activations pre-transposed as bf16 (halves HBM
traffic vs f32 — a pure layout/dtype staging choice).

Algebra (5 projections instead of 6):
  kd  = (k0 - k1) @ Wk.T          (subtract raw views first; projection is linear)
  vd  = (v0 - v1) @ Wk.T
  a0  = sigmoid(scale * sum_head(qhat * kd))        (2-view softmax == sigmoid)
  out = v1 @ W2.T + (a0 (.)head vd) @ Wp.T + bp,    W2 = Wp @ Wk  (host-folded)

The q/kd projections only feed the logits (errors are sigmoid-damped), so they
run in fp8e4 with DoubleRow perf mode (2 contraction rows per PE cell = 2x
throughput). Weights for those two projections are host-scaled by 256 to clear
the fp8e4 subnormal range; the 1/65536 compensation folds into the sigmoid
scale. The value path (vd / v1 / output proj) stays bf16.

Per-token attention over the 2 views needs only per-head segmented reductions
(hm mask matmul) and a per-head broadcast of a0 (sel matmul), both tiny on PE.
"""

from contextlib import ExitStack

import numpy as np
import ml_dtypes

import concourse.bass as bass
import concourse.mybir as mybir
import concourse.tile as tile
from concourse import bacc
from concourse.bass_utils import run_bass_kernel_spmd

B, N, C, NH, HD = 8, 4096, 768, 8, 96
P = 128
KO = C // P            # 6 channel chunks of 128
KO2 = KO // 2          # 3 double-row chunks of 256
BLK = 512              # tokens per block
NBLK = N // BLK        # 8 blocks per core
NCORES = 8
SCALE = float(HD) ** -0.5
W8SCALE = 256.0        # fp8 weight prescale (host); folded out of the sigmoid
SCALE_SIG = SCALE / (W8SCALE * W8SCALE)
F32 = mybir.dt.float32
BF16 = mybir.dt.bfloat16
FP8 = mybir.dt.float8e4
DR = mybir.MatmulPerfMode.DoubleRow

_STATE = {}


def _build_core_kernel(ctx, tc, aps, reps=1, loop_n=1):
    nc = tc.nc

    consts = ctx.enter_context(tc.tile_pool(name="consts", bufs=1))
    inp = ctx.enter_context(tc.tile_pool(name="inp", bufs=2))
    att = ctx.enter_context(tc.tile_pool(name="att", bufs=2))
    # PSUM: 8 banks total. qk: rotating q/kd projection pairs + a0 broadcast
    # (4) | big: vd projection + output projection (3) | psl: logits (1).
    ps_qk = ctx.enter_context(tc.tile_pool(name="ps_qk", bufs=4, space="PSUM"))
    ps_big = ctx.enter_context(tc.tile_pool(name="ps_big", bufs=3, space="PSUM"))
    ps_l = ctx.enter_context(tc.tile_pool(name="ps_l", bufs=1, space="PSUM"))

    # Weights land as [P, KO(c_in), C(c_out)]; fp8 ones are host-prescaled
    # and split hi/lo (w ~= hi + lo, both fp8) so only the fp8 *activation*
    # quantization error survives — halves the logits noise for 2 extra
    # DoubleRow matmul groups per projection.
    w_sb = {}
    for wname, dt in (("wq8h", FP8), ("wq8l", FP8), ("wk8h", FP8),
                      ("wk8l", FP8), ("wkT", BF16), ("w2T", BF16),
                      ("wpT", BF16)):
        w_sb[wname] = consts.tile([P, KO, C], dt, tag=wname, name=wname)

    def _load_w(wname):
        nc.sync.dma_start(
            out=w_sb[wname][:],
            in_=aps[wname].rearrange("(ko p) o -> p ko o", p=P),
        )

    # Early consts: everything phase_a of block 0 touches.
    _load_w("wq8h")
    _load_w("wq8l")
    _load_w("wk8h")
    _load_w("wk8l")
    _load_w("wkT")
    bias_sb = consts.tile([P, KO], F32, tag="bias")
    hm_sb = consts.tile([P, KO, NH], BF16, tag="hm")
    sel_sb = consts.tile([NH, KO, P], BF16, tag="sel")
    nc.sync.dma_start(hm_sb[:], aps["hm"])

    def _load_late_consts():
        # consumed only by phase_b1/b2, which are emitted after this point
        _load_w("w2T")
        _load_w("wpT")
        nc.sync.dma_start(bias_sb[:], aps["bias"])
        nc.sync.dma_start(sel_sb[:], aps["sel"])

    r_in = {name: aps[name].rearrange("(ko p) n -> p ko n", p=P)
            for name in ("qT", "k0T", "k1T", "v0T", "v1T")}
    outT_r = aps["outT"].rearrange("(ko p) n -> p ko n", p=P)

    # Logits accumulator: single bank reused by every block; the tile
    # framework's writer-after-reader dep on the sigmoid keeps it correct
    # (the sigmoid of block b is emitted before block b+1's hm matmuls).
    psl = ps_l.tile([NH, BLK], F32, tag="psl", name="psl")

    def phase_a(blk):
        """Loads, fp8 q/kd projections + logits, bf16 vd projection, sigmoid."""
        ts = bass.ts(blk, BLK)
        q_in = inp.tile([P, KO, BLK], BF16, tag="q", name="q")
        nc.sync.dma_start(out=q_in[:], in_=r_in["qT"][:, :, ts])
        k0 = inp.tile([P, KO, BLK], BF16, tag="k0", name="k0")
        nc.sync.dma_start(out=k0[:], in_=r_in["k0T"][:, :, ts])
        k1 = inp.tile([P, KO, BLK], BF16, tag="k1", name="k1")
        nc.sync.dma_start(out=k1[:], in_=r_in["k1T"][:, :, ts])
        v0 = inp.tile([P, KO, BLK], BF16, tag="v0", name="v0")
        nc.sync.dma_start(out=v0[:], in_=r_in["v0T"][:, :, ts])
        # v1 is read by phase_b2(b), which is emitted after phase_a(b+2):
        # bufs=3 keeps the WAR dep acyclic (bufs=2 would deadlock PE).
        v1 = inp.tile([P, KO, BLK], BF16, tag="v1", name="v1", bufs=3)
        nc.sync.dma_start(out=v1[:], in_=r_in["v1T"][:, :, ts])

        q8 = att.tile([P, KO, BLK], FP8, tag="q8", name="q8")
        nc.scalar.copy(q8[:], q_in[:])
        kd8 = att.tile([P, KO, BLK], FP8, tag="kd8", name="kd8")
        nc.vector.tensor_sub(kd8[:], k0[:], k1[:])
        vd = att.tile([P, KO, BLK], BF16, tag="vd", name="vd")
        nc.vector.tensor_sub(vd[:], v0[:], v1[:])

        # fp8 DoubleRow q/kd projections; qkd = qhat .* kdhat (scaled 65536x).
        # kdhat bounces through SBUF (ACT copy): an engine op may read at
        # most one PSUM operand (NCC_IBVF027).
        qkd = att.tile([P, KO, BLK], BF16, tag="qkd", name="qkd")
        kdh = att.tile([P, KO, BLK], BF16, tag="kdh", name="kdh")
        for oc in range(KO):
            pq = ps_qk.tile([P, BLK], F32, tag="qk", name="pq")
            for i, wn in enumerate(("wq8h", "wq8l")):
                for k2 in range(KO2):
                    nc.tensor.matmul(
                        pq[:], w_sb[wn][:, bass.ts(k2, 2), bass.ts(oc, P)],
                        q8[:, bass.ts(k2, 2), :],
                        start=(i == 0 and k2 == 0),
                        stop=(i == 1 and k2 == KO2 - 1), perf_mode=DR,
                    )
            pk = ps_qk.tile([P, BLK], F32, tag="qk", name="pk")
            for i, wn in enumerate(("wk8h", "wk8l")):
                for k2 in range(KO2):
                    nc.tensor.matmul(
                        pk[:], w_sb[wn][:, bass.ts(k2, 2), bass.ts(oc, P)],
                        kd8[:, bass.ts(k2, 2), :],
                        start=(i == 0 and k2 == 0),
                        stop=(i == 1 and k2 == KO2 - 1), perf_mode=DR,
                    )
            nc.scalar.copy(kdh[:, oc, :], pk[:])
            nc.vector.tensor_mul(qkd[:, oc, :], pq[:], kdh[:, oc, :])

        # logits diff: psl[h, n] = sum_c qkd[c, n] over head h (= 65536*(l0-l1))
        for oc in range(KO):
            nc.tensor.matmul(
                psl[:], hm_sb[:, oc, :], qkd[:, oc, :],
                start=(oc == 0), stop=(oc == KO - 1),
            )

        # vd projection (bf16)
        vdh = att.tile([P, KO, BLK], BF16, tag="vdh", name="vdh")
        for oc in range(KO):
            pv = ps_big.tile([P, BLK], F32, tag="big", name="pv")
            for ko in range(KO):
                nc.tensor.matmul(
                    pv[:], w_sb["wkT"][:, ko, bass.ts(oc, P)], vd[:, ko, :],
                    start=(ko == 0), stop=(ko == KO - 1),
                )
            nc.scalar.copy(vdh[:, oc, :], pv[:])

        # a0 = sigmoid(scale * (l0 - l1)); emitted here (not b1) so the next
        # block's hm matmuls only wait on completed ACT work.
        a = att.tile([NH, BLK], BF16, tag="a", name="a")
        nc.scalar.activation(a[:], psl[:],
                             mybir.ActivationFunctionType.Sigmoid,
                             scale=SCALE_SIG)
        return blk, a, vdh, v1

    def phase_b1(state):
        """Per-head broadcast of a0 (PE) and weighted combine (DVE)."""
        blk, a, vdh, v1 = state
        y_in = att.tile([P, KO, BLK], BF16, tag="y", name="y")
        for oc in range(KO):
            bc = ps_qk.tile([P, BLK], F32, tag="qk", name="bc")
            nc.tensor.matmul(bc[:], sel_sb[:, oc, :], a[:],
                             start=True, stop=True)
            nc.vector.tensor_mul(y_in[:, oc, :], bc[:], vdh[:, oc, :])
        return blk, y_in, v1

    def phase_b2(state):
        """Output projection: v1 @ W2.T + y_in @ Wp.T + bias, store."""
        blk, y_in, v1 = state
        out_sb = att.tile([P, KO, BLK], BF16, tag="out", name="out_sb")
        for oc in range(KO):
            po = ps_big.tile([P, BLK], F32, tag="big", name="po")
            for ko in range(KO):
                nc.tensor.matmul(
                    po[:], w_sb["w2T"][:, ko, bass.ts(oc, P)], v1[:, ko, :],
                    start=(ko == 0), stop=False,
                )
            for ko in range(KO):
                nc.tensor.matmul(
                    po[:], w_sb["wpT"][:, ko, bass.ts(oc, P)], y_in[:, ko, :],
                    start=False, stop=(ko == KO - 1),
                )
            nc.vector.tensor_scalar_add(out_sb[:, oc, :], po[:],
                                        bias_sb[:, bass.ts(oc, 1)])
        nc.sync.dma_start(out=outT_r[:, :, bass.ts(blk, BLK)], in_=out_sb[:])

    # 3-stage software pipeline (same skew as the v1 kernel): per-block PE
    # order is ... A(b+2) | P-proj(b) | bc(b+1) ... so DVE/ACT latencies hide
    # under other blocks' matmuls.
    def emit_pipeline():
        st_a = [phase_a(0)]
        _load_late_consts()
        st_a.append(phase_a(1))
        st_b = [phase_b1(st_a[0])]
        blocks = [(rep, blk) for rep in range(reps) for blk in range(NBLK)]
        for _, blk in blocks[2:]:
            st_a.append(phase_a(blk))
            phase_b2(st_b[-1])
            st_b.append(phase_b1(st_a[-2]))
        phase_b2(st_b[-1])
        st_b.append(phase_b1(st_a[-1]))
        phase_b2(st_b[-1])

    if loop_n > 1:
        # hardware loop: same program size for any repeat count (device-side
        # timing — host launch jitter is ~10-70ms, far above kernel time)
        with tc.For_i(0, loop_n):
            emit_pipeline()
    else:
        emit_pipeline()


def build_program(reps=1, loop_n=1):
    nc = bacc.Bacc("TRN2", debug=False, target_bir_lowering=False)
    aps = {}
    for name in ("qT", "k0T", "k1T", "v0T", "v1T"):
        aps[name] = nc.dram_tensor(name, [C, N], BF16, kind="ExternalInput").ap()
    for name in ("wq8h", "wq8l", "wk8h", "wk8l"):
        aps[name] = nc.dram_tensor(name, [C, C], FP8, kind="ExternalInput").ap()
    for name in ("wkT", "w2T", "wpT"):
        aps[name] = nc.dram_tensor(name, [C, C], BF16, kind="ExternalInput").ap()
    aps["bias"] = nc.dram_tensor("bias", [P, KO], F32, kind="ExternalInput").ap()
    aps["hm"] = nc.dram_tensor("hm", [P, KO, NH], BF16, kind="ExternalInput").ap()
    aps["sel"] = nc.dram_tensor("sel", [NH, KO, P], BF16, kind="ExternalInput").ap()
    aps["outT"] = nc.dram_tensor("outT", [C, N], BF16, kind="ExternalOutput").ap()

    with tile.TileContext(nc) as tc, ExitStack() as ctx:
        _build_core_kernel(ctx, tc, aps, reps=reps, loop_n=loop_n)
    nc.compile()
    return nc


def _get_program():
    if "nc" not in _STATE:
        _STATE["nc"] = build_program()
    return _STATE["nc"]


def make_host_constants(bp):
    bf = ml_dtypes.bfloat16
    heads = np.arange(C) // HD                      # [C]
    bias = np.ascontiguousarray(
        np.asarray(bp, np.float32).reshape(KO, P).T)  # [P, KO]
    hm = np.zeros((C, NH), np.float32)
    for h in range(NH):
        hm[heads == h, h] = 1.0
    hm = np.ascontiguousarray(
        hm.reshape(KO, P, NH).transpose(1, 0, 2)).astype(bf)  # [P, KO, NH]
    sel = np.zeros((NH, C), np.float32)
    for h in range(NH):
        sel[h, heads == h] = 1.0
    sel = np.ascontiguousarray(
        sel.reshape(NH, KO, P)).astype(bf)           # [NH, KO, P]
    return bias, hm, sel


def make_in_maps(query, key, value, Wq, Wk, Wp, bp):
    bf = ml_dtypes.bfloat16
    f8 = ml_dtypes.float8_e4m3
    query = np.asarray(query, np.float32)
    key = np.asarray(key, np.float32)
    value = np.asarray(value, np.float32)
    Wq = np.asarray(Wq, np.float32)
    Wk = np.asarray(Wk, np.float32)
    Wp = np.asarray(Wp, np.float32)
    def split8(w):
        ws = np.ascontiguousarray(np.clip(w.T * W8SCALE, -240.0, 240.0))
        hi = ws.astype(f8)
        lo = (ws - hi.astype(np.float32)).astype(f8)
        return hi, lo

    wq8h, wq8l = split8(Wq)
    wk8h, wk8l = split8(Wk)
    wkT = np.ascontiguousarray(Wk.T).astype(bf)
    w2T = np.ascontiguousarray((Wp @ Wk).T).astype(bf)
    wpT = np.ascontiguousarray(Wp.T).astype(bf)
    bias, hm, sel = make_host_constants(bp)
    in_maps = []
    for b in range(NCORES):
        in_maps.append({
            "qT": np.ascontiguousarray(query[b].T).astype(bf),
            "k0T": np.ascontiguousarray(key[b, :, 0, :].T).astype(bf),
            "k1T": np.ascontiguousarray(key[b, :, 1, :].T).astype(bf),
            "v0T": np.ascontiguousarray(value[b, :, 0, :].T).astype(bf),
            "v1T": np.ascontiguousarray(value[b, :, 1, :].T).astype(bf),
            "wq8h": wq8h, "wq8l": wq8l, "wk8h": wk8h, "wk8l": wk8l,
            "wkT": wkT, "w2T": w2T, "wpT": wpT,
            "bias": bias, "hm": hm, "sel": sel,
        })
    return in_maps


def run(query, key, value, Wq, Wk, Wp, bp, trace=False, **trace_kwargs):
    nc = _get_program()
    in_maps = make_in_maps(query, key, value, Wq, Wk, Wp, bp)
    res = run_bass_kernel_spmd(nc, in_maps, list(range(NCORES)),
                               trace=trace, **trace_kwargs)
    out = np.stack([np.ascontiguousarray(r["outT"]).astype(np.float32).T
                    for r in res.results])
    return out, res


def kernel(query, key, value, Wq, Wk, Wp, bp):
    out, _ = run(query, key, value, Wq, Wk, Wp, bp)
    return out


# revision 22
# speedup vs baseline: 2.0887x; 1.1241x over previous
"""Trainium2 Bass kernel for nn_CrossAttention (B=8, N=4096, C=768, NH=8, 2 views).

Strategy: pure data-parallel over batch B (one batch element per NeuronCore).
Everything runs in "transposed space" (channels on SBUF partitions, tokens on
the free axis). Host stages activations pre-transposed as bf16 (halves HBM
traffic vs f32 — a pure layout/dtype staging choice).

Algebra (5 projections instead of 6):
  kd  = (k0 - k1) @ Wk.T          (subtract raw views first; projection is linear)
  vd  = (v0 - v1) @ Wk.T
  a0  = sigmoid(scale * sum_head(qhat * kd))        (2-view softmax == sigmoid)
  out = v1 @ W2.T + (a0 (.)head vd) @ Wp.T + bp,    W2 = Wp @ Wk  (host-folded)

The q/kd projections only feed the logits (errors are sigmoid-damped), so they
run in fp8e4 with DoubleRow perf mode (2 contraction rows per PE cell = 2x
throughput). Weights for those two projections are host-scaled by 256 to clear
the fp8e4 subnormal range; the 1/65536 compensation folds into the sigmoid
scale. The value path (vd / v1 / output proj) stays bf16.

Per-token attention over the 2 views needs only per-head segmented reductions
(hm mask matmul) and a per-head broadcast of a0 (sel matmul), both tiny on PE.
"""

from contextlib import ExitStack

import numpy as np
import ml_dtypes

import concourse.bass as bass
import concourse.mybir as mybir
import concourse.tile as tile
from concourse import bacc
from concourse.bass_utils import run_bass_kernel_spmd

B, N, C, NH, HD = 8, 4096, 768, 8, 96
P = 128
KO = C // P            # 6 channel chunks of 128
KO2 = KO // 2          # 3 double-row chunks of 256
BLK = 512              # tokens per block
NBLK = N // BLK        # 8 blocks per core
NCORES = 8
SCALE = float(HD) ** -0.5
W8SCALE = 256.0        # fp8 weight prescale (host); folded out of the sigmoid
SCALE_SIG = SCALE / (W8SCALE * W8SCALE)
F32 = mybir.dt.float32
BF16 = mybir.dt.bfloat16
FP8 = mybir.dt.float8e4
DR = mybir.MatmulPerfMode.DoubleRow

_STATE = {}


def _build_core_kernel(ctx, tc, aps, reps=1, loop_n=1):
    nc = tc.nc

    consts = ctx.enter_context(tc.tile_pool(name="consts", bufs=1))
    inp = ctx.enter_context(tc.tile_pool(name="inp", bufs=2))
    att = ctx.enter_context(tc.tile_pool(name="att", bufs=2))
    # PSUM: 8 banks total. qk: rotating q/kd projection pairs + a0 broadcast
    # (4) | big: vd projection + output projection (3) | psl: logits (1).
    ps_qk = ctx.enter_context(tc.tile_pool(name="ps_qk", bufs=4, space="PSUM"))
    ps_big = ctx.enter_context(tc.tile_pool(name="ps_big", bufs=3, space="PSUM"))
    ps_l = ctx.enter_context(tc.tile_pool(name="ps_l", bufs=1, space="PSUM"))

    # Weights land as [P, KO(c_in), C(c_out)], all bf16. (fp8 DoubleRow was
    # tried for the q/kd projections and measured slower: a DoubleRow matmul
    # needs a 256-column LDWEIGHTS that cannot hide behind its own 107ns
    # matmul, so the path is weight-load bound; bf16 128-col loads hide
    # fully behind 213ns matmuls.)
    w_sb = {}
    for wname in ("wqT", "wkT", "w2T", "wpT"):
        w_sb[wname] = consts.tile([P, KO, C], BF16, tag=wname, name=wname)

    def _load_w(wname):
        nc.sync.dma_start(
            out=w_sb[wname][:],
            in_=aps[wname].rearrange("(ko p) o -> p ko o", p=P),
        )

    # Early consts: everything phase_a of block 0 touches.
    _load_w("wqT")
    _load_w("wkT")
    bias_sb = consts.tile([P, KO], F32, tag="bias")
    hm_sb = consts.tile([P, KO, NH], BF16, tag="hm")
    sel_sb = consts.tile([NH, KO, P], BF16, tag="sel")
    nc.sync.dma_start(hm_sb[:], aps["hm"])

    def _load_late_consts():
        # consumed only by phase_b1/b2, which are emitted after this point
        _load_w("w2T")
        _load_w("wpT")
        nc.sync.dma_start(bias_sb[:], aps["bias"])
        nc.sync.dma_start(sel_sb[:], aps["sel"])

    r_in = {name: aps[name].rearrange("(ko p) n -> p ko n", p=P)
            for name in ("qT", "k0T", "k1T", "v0T", "v1T")}
    outT_r = aps["outT"].rearrange("(ko p) n -> p ko n", p=P)

    # Logits accumulator: single bank reused by every block; the tile
    # framework's writer-after-reader dep on the sigmoid keeps it correct
    # (the sigmoid of block b is emitted before block b+1's hm matmuls).
    psl = ps_l.tile([NH, BLK], F32, tag="psl", name="psl")

    def phase_a(blk):
        """Loads, fp8 q/kd projections + logits, bf16 vd projection, sigmoid."""
        ts = bass.ts(blk, BLK)
        q_in = inp.tile([P, KO, BLK], BF16, tag="q", name="q")
        nc.sync.dma_start(out=q_in[:], in_=r_in["qT"][:, :, ts])
        k0 = inp.tile([P, KO, BLK], BF16, tag="k0", name="k0")
        nc.sync.dma_start(out=k0[:], in_=r_in["k0T"][:, :, ts])
        k1 = inp.tile([P, KO, BLK], BF16, tag="k1", name="k1")
        nc.sync.dma_start(out=k1[:], in_=r_in["k1T"][:, :, ts])
        v0 = inp.tile([P, KO, BLK], BF16, tag="v0", name="v0")
        nc.sync.dma_start(out=v0[:], in_=r_in["v0T"][:, :, ts])
        # v1 is read by phase_b2(b), which is emitted after phase_a(b+2):
        # bufs=3 keeps the WAR dep acyclic (bufs=2 would deadlock PE).
        v1 = inp.tile([P, KO, BLK], BF16, tag="v1", name="v1", bufs=3)
        nc.sync.dma_start(out=v1[:], in_=r_in["v1T"][:, :, ts])

        kd = att.tile([P, KO, BLK], BF16, tag="kd", name="kd")
        nc.vector.tensor_sub(kd[:], k0[:], k1[:])
        vd = att.tile([P, KO, BLK], BF16, tag="vd", name="vd")
        nc.vector.tensor_sub(vd[:], v0[:], v1[:])

        # q/kd projections (bf16); qkd = qhat .* kdhat. kdhat bounces through
        # SBUF (ACT copy): an engine op may read at most one PSUM operand
        # (NCC_IBVF027).
        qkd = att.tile([P, KO, BLK], BF16, tag="qkd", name="qkd")
        kdh = att.tile([P, KO, BLK], BF16, tag="kdh", name="kdh")
        for oc in range(KO):
            pq = ps_qk.tile([P, BLK], F32, tag="qk", name="pq")
            for ko in range(KO):
                nc.tensor.matmul(
                    pq[:], w_sb["wqT"][:, ko, bass.ts(oc, P)], q_in[:, ko, :],
                    start=(ko == 0), stop=(ko == KO - 1),
                )
            pk = ps_qk.tile([P, BLK], F32, tag="qk", name="pk")
            for ko in range(KO):
                nc.tensor.matmul(
                    pk[:], w_sb["wkT"][:, ko, bass.ts(oc, P)], kd[:, ko, :],
                    start=(ko == 0), stop=(ko == KO - 1),
                )
            nc.scalar.copy(kdh[:, oc, :], pk[:])
            nc.vector.tensor_mul(qkd[:, oc, :], pq[:], kdh[:, oc, :])

        # logits diff: psl[h, n] = sum_c qkd[c, n] over head h (= 65536*(l0-l1))
        for oc in range(KO):
            nc.tensor.matmul(
                psl[:], hm_sb[:, oc, :], qkd[:, oc, :],
                start=(oc == 0), stop=(oc == KO - 1),
            )

        # vd projection (bf16)
        vdh = att.tile([P, KO, BLK], BF16, tag="vdh", name="vdh")
        for oc in range(KO):
            pv = ps_big.tile([P, BLK], F32, tag="big", name="pv")
            for ko in range(KO):
                nc.tensor.matmul(
                    pv[:], w_sb["wkT"][:, ko, bass.ts(oc, P)], vd[:, ko, :],
                    start=(ko == 0), stop=(ko == KO - 1),
                )
            nc.scalar.copy(vdh[:, oc, :], pv[:])

        # a0 = sigmoid(scale * (l0 - l1)); emitted here (not b1) so the next
        # block's hm matmuls only wait on completed ACT work.
        a = att.tile([NH, BLK], BF16, tag="a", name="a")
        nc.scalar.activation(a[:], psl[:],
                             mybir.ActivationFunctionType.Sigmoid,
                             scale=SCALE)
        return blk, a, vdh, v1

    def phase_b1(state):
        """Per-head broadcast of a0 (PE) and weighted combine (DVE)."""
        blk, a, vdh, v1 = state
        y_in = att.tile([P, KO, BLK], BF16, tag="y", name="y")
        for oc in range(KO):
            bc = ps_qk.tile([P, BLK], F32, tag="qk", name="bc")
            nc.tensor.matmul(bc[:], sel_sb[:, oc, :], a[:],
                             start=True, stop=True)
            nc.vector.tensor_mul(y_in[:, oc, :], bc[:], vdh[:, oc, :])
        return blk, y_in, v1

    def phase_b2(state):
        """Output projection: v1 @ W2.T + y_in @ Wp.T + bias, store."""
        blk, y_in, v1 = state
        out_sb = att.tile([P, KO, BLK], BF16, tag="out", name="out_sb")
        for oc in range(KO):
            po = ps_big.tile([P, BLK], F32, tag="big", name="po")
            for ko in range(KO):
                nc.tensor.matmul(
                    po[:], w_sb["w2T"][:, ko, bass.ts(oc, P)], v1[:, ko, :],
                    start=(ko == 0), stop=False,
                )
            for ko in range(KO):
                nc.tensor.matmul(
                    po[:], w_sb["wpT"][:, ko, bass.ts(oc, P)], y_in[:, ko, :],
                    start=False, stop=(ko == KO - 1),
                )
            nc.vector.tensor_scalar_add(out_sb[:, oc, :], po[:],
                                        bias_sb[:, bass.ts(oc, 1)])
        nc.sync.dma_start(out=outT_r[:, :, bass.ts(blk, BLK)], in_=out_sb[:])

    # 3-stage software pipeline (same skew as the v1 kernel): per-block PE
    # order is ... A(b+2) | P-proj(b) | bc(b+1) ... so DVE/ACT latencies hide
    # under other blocks' matmuls.
    def emit_pipeline():
        st_a = [phase_a(0)]
        _load_late_consts()
        st_a.append(phase_a(1))
        st_b = [phase_b1(st_a[0])]
        blocks = [(rep, blk) for rep in range(reps) for blk in range(NBLK)]
        for _, blk in blocks[2:]:
            st_a.append(phase_a(blk))
            phase_b2(st_b[-1])
            st_b.append(phase_b1(st_a[-2]))
        phase_b2(st_b[-1])
        st_b.append(phase_b1(st_a[-1]))
        phase_b2(st_b[-1])

    if loop_n > 1:
        # hardware loop: same program size for any repeat count (device-side
        # timing — host launch jitter is ~10-70ms, far above kernel time)
        with tc.For_i(0, loop_n):
            emit_pipeline()
    else:
        emit_pipeline()


def build_program(reps=1, loop_n=1):
    nc = bacc.Bacc("TRN2", debug=False, target_bir_lowering=False)
    aps = {}
    for name in ("qT", "k0T", "k1T", "v0T", "v1T"):
        aps[name] = nc.dram_tensor(name, [C, N], BF16, kind="ExternalInput").ap()
    for name in ("wqT", "wkT", "w2T", "wpT"):
        aps[name] = nc.dram_tensor(name, [C, C], BF16, kind="ExternalInput").ap()
    aps["bias"] = nc.dram_tensor("bias", [P, KO], F32, kind="ExternalInput").ap()
    aps["hm"] = nc.dram_tensor("hm", [P, KO, NH], BF16, kind="ExternalInput").ap()
    aps["sel"] = nc.dram_tensor("sel", [NH, KO, P], BF16, kind="ExternalInput").ap()
    aps["outT"] = nc.dram_tensor("outT", [C, N], BF16, kind="ExternalOutput").ap()

    with tile.TileContext(nc) as tc, ExitStack() as ctx:
        _build_core_kernel(ctx, tc, aps, reps=reps, loop_n=loop_n)
    nc.compile()
    return nc


def _get_program():
    if "nc" not in _STATE:
        _STATE["nc"] = build_program()
    return _STATE["nc"]


def make_host_constants(bp):
    bf = ml_dtypes.bfloat16
    heads = np.arange(C) // HD                      # [C]
    bias = np.ascontiguousarray(
        np.asarray(bp, np.float32).reshape(KO, P).T)  # [P, KO]
    hm = np.zeros((C, NH), np.float32)
    for h in range(NH):
        hm[heads == h, h] = 1.0
    hm = np.ascontiguousarray(
        hm.reshape(KO, P, NH).transpose(1, 0, 2)).astype(bf)  # [P, KO, NH]
    sel = np.zeros((NH, C), np.float32)
    for h in range(NH):
        sel[h, heads == h] = 1.0
    sel = np.ascontiguousarray(
        sel.reshape(NH, KO, P)).astype(bf)           # [NH, KO, P]
    return bias, hm, sel


def make_in_maps(query, key, value, Wq, Wk, Wp, bp):
    bf = ml_dtypes.bfloat16
    f8 = ml_dtypes.float8_e4m3
    query = np.asarray(query, np.float32)
    key = np.asarray(key, np.float32)
    value = np.asarray(value, np.float32)
    Wq = np.asarray(Wq, np.float32)
    Wk = np.asarray(Wk, np.float32)
    Wp = np.asarray(Wp, np.float32)
    wqT = np.ascontiguousarray(Wq.T).astype(bf)
    wkT = np.ascontiguousarray(Wk.T).astype(bf)
    w2T = np.ascontiguousarray((Wp @ Wk).T).astype(bf)
    wpT = np.ascontiguousarray(Wp.T).astype(bf)
    bias, hm, sel = make_host_constants(bp)
    in_maps = []
    for b in range(NCORES):
        in_maps.append({
            "qT": np.ascontiguousarray(query[b].T).astype(bf),
            "k0T": np.ascontiguousarray(key[b, :, 0, :].T).astype(bf),
            "k1T": np.ascontiguousarray(key[b, :, 1, :].T).astype(bf),
            "v0T": np.ascontiguousarray(value[b, :, 0, :].T).astype(bf),
            "v1T": np.ascontiguousarray(value[b, :, 1, :].T).astype(bf),
            "wqT": wqT, "wkT": wkT, "w2T": w2T, "wpT": wpT,
            "bias": bias, "hm": hm, "sel": sel,
        })
    return in_maps


def run(query, key, value, Wq, Wk, Wp, bp, trace=False, **trace_kwargs):
    nc = _get_program()
    in_maps = make_in_maps(query, key, value, Wq, Wk, Wp, bp)
    res = run_bass_kernel_spmd(nc, in_maps, list(range(NCORES)),
                               trace=trace, **trace_kwargs)
    out = np.stack([np.ascontiguousarray(r["outT"]).astype(np.float32).T
                    for r in res.results])
    return out, res


def kernel(query, key, value, Wq, Wk, Wp, bp):
    out, _ = run(query, key, value, Wq, Wk, Wp, bp)
    return out
